# revision 1
# baseline (speedup 1.0000x reference)
"""Trainium2 Bass kernel for nn_CorrBlock_cascade (self-contained).

Pipeline (per core, core i handles clip/segment i = frames 8i..8i+7):
  conv21 (1x1, 64->16) -> BN21(relu) -> temporal shift -> 7x7 local corr
  -> BN22(relu) -> conv22 (1x1, 49->64) -> BN23 -> +residual -> relu
BN statistics are all-reduced across the 8 cores.

Device layouts:
  x / z / out : [128 = (f2, c), 3136] per frame-pair (4 pairs)
  y / a / products / corr rounds : [128 = (f, cm), 3136]
  corr2 (conv22 rhs) : [98 = (f2, k), 4*3136]
"""

import os
import numpy as np
import ml_dtypes

import concourse.bacc as bacc
import concourse.bass as bass
import concourse.mybir as mybir
from concourse import tile
from concourse.bass_utils import run_bass_kernel_spmd

N_CORES = 8
NT, C, H, W = 64, 64, 56, 56
CM = C // 4                  # 16
F = NT // N_CORES            # 8 frames per core
P = H * W                    # 3136
WPAD = 62                    # 56 + 2*3
PPAD = WPAD * WPAD           # 3844
BPAD_ALLOC = 3908            # padded alloc so shifted [56,62] views stay in-bounds
KK = 49
NTOT = float(NT * P)         # BN sample count per channel (global)
EPS = 1e-5
NCH = 7
CHUNK = P // NCH             # 448
ROUNDS = [16, 16, 16, 1]     # 49 offsets in 4 matmul-accumulation rounds
DT = mybir.dt
BF16 = ml_dtypes.bfloat16


def _build_nc(dbg=False):
    nc = bacc.Bacc("TRN2", target_bir_lowering=False, debug=False,
                   num_devices=N_CORES)
    dbg_tensors = {}

    def dump(name, sb_tile, shape, dtype):
        if not dbg:
            return
        d = nc.dram_tensor(f"dbg_{name}", shape, dtype, kind="ExternalOutput")
        dbg_tensors[name] = d
        nc.sync.dma_start(d[:], sb_tile)

    x4_d = nc.dram_tensor("x4", [4, 128, P], DT.float32, kind="ExternalInput")
    w21bd_d = nc.dram_tensor("w21bd", [128, 32], DT.bfloat16, kind="ExternalInput")
    w22bd_d = nc.dram_tensor("w22bd", [98, 128], DT.bfloat16, kind="ExternalInput")
    selred_d = nc.dram_tensor("selred", [128, 16 * 128], DT.bfloat16,
                              kind="ExternalInput")
    selb16_d = nc.dram_tensor("selb16", [128, 128], DT.float32, kind="ExternalInput")
    selb64_d = nc.dram_tensor("selb64", [128, 128], DT.float32, kind="ExternalInput")
    selbk_d = nc.dram_tensor("selbk", [128, 4 * 98], DT.float32, kind="ExternalInput")
    bnc128_d = nc.dram_tensor("bnc128", [128, 4], DT.float32, kind="ExternalInput")
    bnc98_d = nc.dram_tensor("bnc98", [98, 2], DT.float32, kind="ExternalInput")
    out_d = nc.dram_tensor("out", [4, 128, P], DT.float32, kind="ExternalOutput")

    RELU = mybir.ActivationFunctionType.Relu
    COPY = mybir.ActivationFunctionType.Copy
    SQRT = mybir.ActivationFunctionType.Sqrt
    MULT = mybir.AluOpType.mult
    ADD = mybir.AluOpType.add

    with tile.TileContext(nc) as tc:
        with (
            tc.tile_pool(name="const", bufs=1) as cpool,
            tc.tile_pool(name="big", bufs=1) as bpool,
            tc.tile_pool(name="work", bufs=1) as wpool,
            tc.tile_pool(name="prod", bufs=3) as ppool,
            tc.tile_pool(name="small", bufs=1) as spool,
            tc.tile_pool(name="psum", bufs=1, space="PSUM") as pspool,
            tc.tile_pool(name="dram", bufs=1, space="DRAM") as dpool,
        ):
            # ---- load constants ----
            w21bd = cpool.tile([128, 32], DT.bfloat16)
            w22bd = cpool.tile([98, 128], DT.bfloat16)
            selred = cpool.tile([128, 16 * 128], DT.bfloat16)
            selb16 = cpool.tile([128, 128], DT.float32)
            selb64 = cpool.tile([128, 128], DT.float32)
            selbk = cpool.tile([128, 4 * 98], DT.float32)
            bnc128 = cpool.tile([128, 4], DT.float32)
            bnc98 = cpool.tile([98, 2], DT.float32)
            for sb_t, dr_t in [(w21bd, w21bd_d), (w22bd, w22bd_d),
                               (selred, selred_d), (selb16, selb16_d),
                               (selb64, selb64_d), (selbk, selbk_d),
                               (bnc128, bnc128_d), (bnc98, bnc98_d)]:
                nc.sync.dma_start(sb_t[:], dr_t[:])

            # ---- load x (fp32 -> bf16 cast in DMA) ----
            x_all = bpool.tile([128, 4 * P], DT.bfloat16, tag="x")
            for p in range(4):
                nc.gpsimd.dma_start(x_all[:, p * P:(p + 1) * P], x4_d[p])

            # ---- conv21: y[(f,cm), pix] ----
            psum_y = pspool.tile([128, NCH, 512], DT.float32, tag="bigps")
            for p in range(4):
                for ch in range(NCH):
                    nc.tensor.matmul(
                        psum_y[32 * p:32 * p + 32, ch, 0:CHUNK],
                        w21bd[:], x_all[:, p * P + ch * CHUNK:p * P + (ch + 1) * CHUNK],
                        start=True, stop=True, tile_position=(0, 32 * p))

            # drain + BN21 stats
            y_sb = wpool.tile([128, P], DT.bfloat16, tag="y")
            st1 = spool.tile([128, 8], DT.float32)
            for ch in range(NCH):
                sl = slice(ch * CHUNK, (ch + 1) * CHUNK)
                nc.scalar.activation(y_sb[:, sl], psum_y[:, ch, 0:CHUNK], COPY,
                                     accum_out=st1[:, ch:ch + 1])
            trash = ppool.tile([128, P], DT.bfloat16, tag="prod")
            nc.vector.scalar_tensor_tensor(trash[:], y_sb[:], 1.0, y_sb[:],
                                           op0=MULT, op1=MULT,
                                           accum_out=st1[:, 7:8])
            dump("y", y_sb[:], [128, P], DT.bfloat16)
            ar1 = spool.tile([128, 2], DT.float32)
            nc.vector.tensor_reduce(ar1[:, 0:1], st1[:, 0:7],
                                    axis=mybir.AxisListType.X, op=ADD)
            nc.vector.tensor_copy(ar1[:, 1:2], st1[:, 7:8])

            # AllReduce #1
            cc1i = dpool.tile([128, 2], DT.float32)
            cc1o = dpool.tile([128, 2], DT.float32, addr_space="Shared")
            nc.sync.dma_start(cc1i[:], ar1[:])
            nc.gpsimd.collective_compute(
                "AllReduce", ADD, replica_groups=[list(range(N_CORES))],
                ins=[cc1i.opt()], outs=[cc1o.opt()])
            ar1r = spool.tile([128, 2], DT.float32)
            nc.sync.dma_start(ar1r[:], cc1o[:])

            # BN21 coefficient vectors (per-partition, (f,cm) layout)
            def bn_vectors(npart, psum_st, gvec, bvec, pool):
                """psum_st [npart,2] = (sum, sumsq); returns (svec, tvec)."""
                mean = pool.tile([npart, 1], DT.float32, name=f"mean{nc.next_id()}")
                e2 = pool.tile([npart, 1], DT.float32, name=f"e2{nc.next_id()}")
                var = pool.tile([npart, 1], DT.float32, name=f"var{nc.next_id()}")
                std = pool.tile([npart, 1], DT.float32, name=f"std{nc.next_id()}")
                rstd = pool.tile([npart, 1], DT.float32, name=f"rstd{nc.next_id()}")
                svec = pool.tile([npart, 1], DT.float32, name=f"svec{nc.next_id()}")
                tv = pool.tile([npart, 1], DT.float32, name=f"tv{nc.next_id()}")
                tvec = pool.tile([npart, 1], DT.float32, name=f"tvec{nc.next_id()}")
                eps_t = pool.tile([npart, 1], DT.float32, name=f"eps{nc.next_id()}")
                nc.vector.memset(eps_t[:], EPS)
                nc.scalar.mul(mean[:], psum_st[:, 0:1], 1.0 / NTOT)
                nc.scalar.mul(e2[:], psum_st[:, 1:2], 1.0 / NTOT)
                nc.vector.tensor_mul(var[:], mean[:], mean[:])
                nc.vector.tensor_sub(var[:], e2[:], var[:])
                nc.scalar.activation(std[:], var[:], SQRT, bias=eps_t[:])
                nc.vector.reciprocal(rstd[:], std[:])
                nc.vector.tensor_mul(svec[:], gvec, rstd[:])
                nc.vector.tensor_mul(tv[:], mean[:], svec[:])
                nc.vector.tensor_sub(tvec[:], bvec, tv[:])
                return svec, tvec

            pst1 = pspool.tile([128, 2], DT.float32, tag="stps")
            nc.tensor.matmul(pst1[:], selb16[:], ar1r[:], start=True, stop=True)
            s21, t21 = bn_vectors(128, pst1, bnc128[:, 0:1], bnc128[:, 1:2], spool)

            dump("ar1r", ar1r[:], [128, 2], DT.float32)
            dump("s21", s21[:], [128, 1], DT.float32)
            dump("t21", t21[:], [128, 1], DT.float32)
            # BN21 apply + relu (in place on y_sb -> "a")
            nc.scalar.activation(y_sb[:], y_sb[:], RELU, bias=t21[:], scale=s21[:])
            a_bf = y_sb

            # ---- build padded, temporally-shifted b (and odd-parity copy) ----
            bpad = wpool.tile([128, BPAD_ALLOC], DT.bfloat16, tag="bpad")
            bpad1 = wpool.tile([128, BPAD_ALLOC], DT.bfloat16, tag="bpad1")
            nc.vector.memset(bpad[:], 0.0)
            nc.vector.memset(bpad1[:], 0.0)

            def interior(t, shift):
                # AP over [(y+3)*62 + (x+3) - shift] for y,x in [0,56)
                base = 3 * WPAD + 3 - shift
                v = t[:, base:base + 56 * WPAD]
                v = v.rearrange("p (y x) -> p y x", y=56, x=WPAD)
                return v[:, :, 0:56]

            # b frame f = a frame f+1 (last frame pairs with itself)
            a3d = a_bf[:].rearrange("p (y x) -> p y x", y=56, x=56)
            nc.sync.dma_start(interior(bpad, 0)[0:112], a3d[16:128])
            nc.sync.dma_start(interior(bpad, 0)[112:128], a3d[112:128])
            nc.sync.dma_start(interior(bpad1, 1)[0:112], a3d[16:128])
            nc.sync.dma_start(interior(bpad1, 1)[112:128], a3d[112:128])

            dump("a", a_bf[:], [128, P], DT.bfloat16)
            dump("bpad", bpad[:], [128, BPAD_ALLOC], DT.bfloat16)
            dump("bpad1", bpad1[:], [128, BPAD_ALLOC], DT.bfloat16)
            # ---- correlation: 49 shifted multiplies + PE group-reduce ----
            corr_all = bpool.tile([128, 4 * P], DT.bfloat16, tag="corr")
            st2 = spool.tile([128, 8], DT.float32)
            a3dv = a_bf[:].rearrange("p (y x) -> p y x", y=56, x=56)
            k = 0
            for r, nslots in enumerate(ROUNDS):
                psum_corr = pspool.tile([128, NCH, 512], DT.float32, tag="bigps",
                                        name=f"psc{r}")
                for s in range(nslots):
                    dy, dx = k // 7, k % 7
                    delta = WPAD * dy + dx
                    src, off = (bpad, delta) if delta % 2 == 0 else (bpad1, delta - 1)
                    bview = src[:, off:off + 56 * WPAD]
                    bview = bview.rearrange("p (y x) -> p y x", y=56, x=WPAD)
                    bview = bview[:, :, 0:56]
                    prod = ppool.tile([128, P], DT.bfloat16, tag="prod",
                                      name=f"prod{k}")
                    p3d = prod[:].rearrange("p (y x) -> p y x", y=56, x=56)
                    nc.vector.tensor_mul(p3d, a3dv, bview)
                    for ch in range(NCH):
                        sl = slice(ch * CHUNK, (ch + 1) * CHUNK)
                        nc.tensor.matmul(
                            psum_corr[:, ch, 0:CHUNK],
                            selred[:, 128 * s:128 * (s + 1)],
                            prod[:, sl],
                            start=(s == 0), stop=(s == nslots - 1))
                    k += 1
                # drain round r + BN22 stats
                csl = slice(r * P, (r + 1) * P)
                corr_v = corr_all[:, csl].rearrange("p (c x) -> p c x",
                                                    c=NCH, x=CHUNK)
                nc.scalar.activation(corr_v, psum_corr[:, :, 0:CHUNK], COPY,
                                     accum_out=st2[:, r:r + 1])
                trash2 = ppool.tile([128, P], DT.bfloat16, tag="prod",
                                    name=f"trash2_{r}")
                nc.vector.scalar_tensor_tensor(
                    trash2[:], corr_all[:, csl], 1.0, corr_all[:, csl],
                    op0=MULT, op1=MULT, accum_out=st2[:, 4 + r:5 + r])

            dump("corr", corr_all[:], [128, 4 * P], DT.bfloat16)
            dump("st2", st2[:], [128, 8], DT.float32)
            # AllReduce #2 (launch) + corr re-layout DMA (overlaps AR latency)
            cc2i = dpool.tile([128, 8], DT.float32)
            cc2o = dpool.tile([128, 8], DT.float32, addr_space="Shared")
            nc.sync.dma_start(cc2i[:], st2[:])
            nc.gpsimd.collective_compute(
                "AllReduce", ADD, replica_groups=[list(range(N_CORES))],
                ins=[cc2i.opt()], outs=[cc2o.opt()])
            ar2r = spool.tile([128, 8], DT.float32)
            nc.sync.dma_start(ar2r[:], cc2o[:])

            corr2 = bpool.tile([98, 4 * P], DT.bfloat16, tag="corr2")
            for r, nslots in enumerate(ROUNDS):
                for f in range(F):
                    src = corr_all[f:8 * (nslots - 1) + f + 1:8,
                                   r * P:(r + 1) * P]
                    dst = corr2[49 * (f % 2) + 16 * r:
                                49 * (f % 2) + 16 * r + nslots,
                                (f // 2) * P:(f // 2 + 1) * P]
                    nc.sync.dma_start(dst, src)

            dump("corr2pre", corr2[:], [98, 4 * P], DT.bfloat16)
            # BN22 vectors in (f2,k) layout
            pst2 = pspool.tile([98, 2], DT.float32, tag="stps", name="pst2")
            ar2v = ar2r[:].rearrange("p (s r) -> p r s", s=2, r=4)
            for r in range(4):
                nc.tensor.matmul(pst2[:], selbk[:, 98 * r:98 * (r + 1)],
                                 ar2v[:, r, :], start=(r == 0), stop=(r == 3))
            s22, t22 = bn_vectors(98, pst2, bnc98[:, 0:1], bnc98[:, 1:2], spool)

            # BN22 apply + relu, in place on corr2
            nc.scalar.activation(corr2[:], corr2[:], RELU,
                                 bias=t22[:], scale=s22[:])

            dump("s22", s22[:], [98, 1], DT.float32)
            dump("t22", t22[:], [98, 1], DT.float32)
            dump("corr2post", corr2[:], [98, 4 * P], DT.bfloat16)
            # ---- conv22 ----
            z_all = bpool.tile([128, 4 * P], DT.bfloat16, tag="z")
            st3 = spool.tile([128, 8], DT.float32)
            for p in range(4):
                psum_z = pspool.tile([128, NCH, 512], DT.float32, tag="bigps",
                                     name=f"psz{p}")
                for ch in range(NCH):
                    nc.tensor.matmul(
                        psum_z[:, ch, 0:CHUNK], w22bd[:],
                        corr2[:, p * P + ch * CHUNK:p * P + (ch + 1) * CHUNK],
                        start=True, stop=True)
                zsl = slice(p * P, (p + 1) * P)
                z_v = z_all[:, zsl].rearrange("p (c x) -> p c x", c=NCH, x=CHUNK)
                nc.scalar.activation(z_v, psum_z[:, :, 0:CHUNK], COPY,
                                     accum_out=st3[:, p:p + 1])
                trash3 = ppool.tile([128, P], DT.bfloat16, tag="prod",
                                    name=f"trash3_{p}")
                nc.vector.scalar_tensor_tensor(
                    trash3[:], z_all[:, zsl], 1.0, z_all[:, zsl],
                    op0=MULT, op1=MULT, accum_out=st3[:, 4 + p:5 + p])

            ar3 = spool.tile([128, 2], DT.float32)
            nc.vector.tensor_reduce(ar3[:, 0:1], st3[:, 0:4],
                                    axis=mybir.AxisListType.X, op=ADD)
            nc.vector.tensor_reduce(ar3[:, 1:2], st3[:, 4:8],
                                    axis=mybir.AxisListType.X, op=ADD)

            # AllReduce #3
            cc3i = dpool.tile([128, 2], DT.float32)
            cc3o = dpool.tile([128, 2], DT.float32, addr_space="Shared")
            nc.sync.dma_start(cc3i[:], ar3[:])
            nc.gpsimd.collective_compute(
                "AllReduce", ADD, replica_groups=[list(range(N_CORES))],
                ins=[cc3i.opt()], outs=[cc3o.opt()])
            ar3r = spool.tile([128, 2], DT.float32)
            nc.sync.dma_start(ar3r[:], cc3o[:])

            pst3 = pspool.tile([128, 2], DT.float32, tag="stps", name="pst3")
            nc.tensor.matmul(pst3[:], selb64[:], ar3r[:], start=True, stop=True)
            s23, t23 = bn_vectors(128, pst3, bnc128[:, 2:3], bnc128[:, 3:4], spool)

            dump("z", z_all[:], [128, 4 * P], DT.bfloat16)
            dump("s23", s23[:], [128, 1], DT.float32)
            dump("t23", t23[:], [128, 1], DT.float32)
            # ---- final: relu(s23*z + t23 + x) ----
            for p in range(4):
                zsl = slice(p * P, (p + 1) * P)
                tmp = ppool.tile([128, P], DT.bfloat16, tag="prod",
                                 name=f"fin{p}")
                nc.vector.scalar_tensor_tensor(
                    tmp[:], z_all[:, zsl], s23[:], x_all[:, zsl],
                    op0=MULT, op1=ADD)
                o32 = wpool.tile([128, P], DT.float32, tag="o32",
                                 name=f"o32_{p}")
                nc.scalar.activation(o32[:], tmp[:], RELU, bias=t23[:])
                nc.sync.dma_start(out_d[p], o32[:])

    nc.compile()
    nc._dbg_names = list(dbg_tensors)
    return nc


def _host_constants(w21, w22):
    w21bd = np.zeros((128, 32), BF16)
    for f2 in range(2):
        w21bd[64 * f2:64 * f2 + 64, 16 * f2:16 * f2 + 16] = w21.T.astype(BF16)
    w22bd = np.zeros((98, 128), BF16)
    for f2 in range(2):
        w22bd[49 * f2:49 * f2 + 49, 64 * f2:64 * f2 + 64] = w22.T.astype(BF16)

    selred = np.zeros((128, 16, 128), BF16)
    for s in range(16):
        for f in range(F):
            selred[16 * f:16 * f + 16, s, 8 * s + f] = 1.0 / CM
    selred = selred.reshape(128, 16 * 128)

    pidx = np.arange(128)
    selb16 = (pidx[:, None] % 16 == pidx[None, :] % 16).astype(np.float32)
    selb64 = (pidx[:, None] % 64 == pidx[None, :] % 64).astype(np.float32)

    selbk = np.zeros((4, 128, 98), np.float32)
    k = 0
    for r, nslots in enumerate(ROUNDS):
        for s in range(nslots):
            for f in range(F):
                for f2 in range(2):
                    selbk[r, 8 * s + f, 49 * f2 + 16 * r + s] = 1.0
            k += 1
    selbk = selbk.transpose(1, 0, 2).reshape(128, 4 * 98)
    return w21bd, w22bd, selred, selb16, selb64, selbk


_NC_CACHE = {}


def kernel(x, w21, w22, g21, b21, g22, b22, g23, b23, trace=False, dbg=False):
    x = np.asarray(x, np.float32)
    w21 = np.asarray(w21, np.float32)
    w22 = np.asarray(w22, np.float32)
    g21 = np.asarray(g21, np.float32); b21 = np.asarray(b21, np.float32)
    g22 = np.asarray(g22, np.float32); b22 = np.asarray(b22, np.float32)
    g23 = np.asarray(g23, np.float32); b23 = np.asarray(b23, np.float32)

    key = ("nc_dbg" if dbg else "nc")
    if key not in _NC_CACHE:
        _NC_CACHE[key] = _build_nc(dbg=dbg)
    nc = _NC_CACHE[key]

    w21bd, w22bd, selred, selb16, selb64, selbk = _host_constants(w21, w22)
    pidx = np.arange(128)
    bnc128 = np.stack([g21[pidx % 16], b21[pidx % 16],
                       g23[pidx % 64], b23[pidx % 64]], 1).astype(np.float32)
    kidx = np.arange(98) % 49
    bnc98 = np.stack([g22[kidx], b22[kidx]], 1).astype(np.float32)

    in_maps = []
    for i in range(N_CORES):
        x4 = np.ascontiguousarray(
            x[F * i:F * (i + 1)].reshape(4, 128, P), np.float32)
        in_maps.append({
            "x4": x4, "w21bd": w21bd, "w22bd": w22bd, "selred": selred,
            "selb16": selb16, "selb64": selb64, "selbk": selbk,
            "bnc128": bnc128, "bnc98": bnc98,
        })

    res = run_bass_kernel_spmd(nc, in_maps, core_ids=list(range(N_CORES)),
                               trace=trace)
    out = np.empty((NT, C, H, W), np.float32)
    for i in range(N_CORES):
        out[F * i:F * (i + 1)] = res.results[i]["out"].reshape(F, C, H, W)
    if dbg:
        return out, res
    if trace:
        return out, res
    return out



# revision 8
# speedup vs baseline: 1.1306x; 1.1306x over previous
"""Trainium2 Bass kernel for nn_CorrBlock_cascade (self-contained).

Pipeline (per core, core i handles clip/segment i = frames 8i..8i+7):
  conv21 (1x1, 64->16) -> BN21(relu) -> temporal shift -> 7x7 local corr
  -> BN22(relu) -> conv22 (1x1, 49->64) -> BN23 -> +residual -> relu
BN statistics are all-reduced across the 8 cores.

v2 changes vs baseline:
  - bf16 input/output (host casts), halving HBM traffic
  - temporal shift built by a PE permutation matmul + Act drain into the
    padded tile (replaces a 25k-descriptor DMA storm), overlapped with
    the BN21 all-reduce
  - corr reduce uses [128,32] stationaries at 4 PE column-tile positions
    (4x smaller LDWEIGHTS)
  - sum-of-squares stats run on gpsimd; corr re-layout DMA per round
  - BN22 scale folded into conv22 weights; bias+relu split DVE/Act
"""

import numpy as np
import ml_dtypes

import concourse.bacc as bacc
import concourse.mybir as mybir
from concourse import tile
from concourse.bass_utils import run_bass_kernel_spmd

N_CORES = 8
NT, C, H, W = 64, 64, 56, 56
CM = C // 4                  # 16
F = NT // N_CORES            # 8 frames per core
P = H * W                    # 3136
WPAD = 62                    # 56 + 2*3
BPAD_ALLOC = 3908
KK = 49
NTOT = float(NT * P)
EPS = 1e-5
NCH = 7
CHUNK = P // NCH             # 448
ROUNDS = [16, 16, 16, 1]
DT = mybir.dt
BF16 = ml_dtypes.bfloat16


def _build_nc(dbg=False):
    nc = bacc.Bacc("TRN2", target_bir_lowering=False, debug=False,
                   num_devices=N_CORES)
    dbg_tensors = {}

    def dump(name, sb_tile, shape, dtype):
        if not dbg:
            return
        d = nc.dram_tensor(f"dbg_{name}", shape, dtype, kind="ExternalOutput")
        dbg_tensors[name] = d
        nc.sync.dma_start(d[:], sb_tile)

    x4_d = nc.dram_tensor("x4", [4, 128, P], DT.bfloat16, kind="ExternalInput")
    w21bd_d = nc.dram_tensor("w21bd", [128, 32], DT.bfloat16, kind="ExternalInput")
    w22bd_d = nc.dram_tensor("w22bd", [98, 128], DT.bfloat16, kind="ExternalInput")
    sel32_d = nc.dram_tensor("sel32", [128, 4 * 32], DT.bfloat16,
                             kind="ExternalInput")
    shiftm_d = nc.dram_tensor("shiftm", [128, 128], DT.bfloat16,
                              kind="ExternalInput")
    selb16_d = nc.dram_tensor("selb16", [128, 128], DT.float32, kind="ExternalInput")
    selb64_d = nc.dram_tensor("selb64", [128, 128], DT.float32, kind="ExternalInput")
    selbk_d = nc.dram_tensor("selbk", [128, 4 * 98], DT.float32, kind="ExternalInput")
    bnc128_d = nc.dram_tensor("bnc128", [128, 4], DT.float32, kind="ExternalInput")
    bnc98_d = nc.dram_tensor("bnc98", [98, 2], DT.float32, kind="ExternalInput")
    out_d = nc.dram_tensor("out", [4, 128, P], DT.bfloat16, kind="ExternalOutput")

    RELU = mybir.ActivationFunctionType.Relu
    COPY = mybir.ActivationFunctionType.Copy
    SQRT = mybir.ActivationFunctionType.Sqrt
    MULT = mybir.AluOpType.mult
    ADD = mybir.AluOpType.add
    MAX = mybir.AluOpType.max

    with tile.TileContext(nc) as tc:
        with (
            tc.tile_pool(name="const", bufs=1) as cpool,
            tc.tile_pool(name="big", bufs=1) as bpool,
            tc.tile_pool(name="work", bufs=1) as wpool,
            tc.tile_pool(name="prod", bufs=3) as ppool,
            tc.tile_pool(name="trash", bufs=2) as tpool,
            tc.tile_pool(name="small", bufs=1) as spool,
            tc.tile_pool(name="psum", bufs=1, space="PSUM") as pspool,
            tc.tile_pool(name="dram", bufs=1, space="DRAM") as dpool,
        ):
            # ---- constants ----
            w21bd = cpool.tile([128, 32], DT.bfloat16)
            w22bd = cpool.tile([98, 128], DT.bfloat16)
            sel32 = cpool.tile([128, 4 * 32], DT.bfloat16)
            shiftm = cpool.tile([128, 128], DT.bfloat16)
            selb16 = cpool.tile([128, 128], DT.float32)
            selb64 = cpool.tile([128, 128], DT.float32)
            selbk = cpool.tile([128, 4 * 98], DT.float32)
            bnc128 = cpool.tile([128, 4], DT.float32)
            bnc98 = cpool.tile([98, 2], DT.float32)
            for sb_t, dr_t in [(w21bd, w21bd_d), (w22bd, w22bd_d),
                               (sel32, sel32_d), (shiftm, shiftm_d),
                               (selb16, selb16_d), (selb64, selb64_d),
                               (selbk, selbk_d), (bnc128, bnc128_d),
                               (bnc98, bnc98_d)]:
                nc.sync.dma_start(sb_t[:], dr_t[:])

            # ---- load x (already bf16) ----
            x_all = bpool.tile([128, 4 * P], DT.bfloat16, tag="x")
            for p in range(4):
                nc.gpsimd.dma_start(x_all[:, p * P:(p + 1) * P], x4_d[p])

            # padded b tiles (zeroed; interiors filled later)
            bpadE = wpool.tile([128, BPAD_ALLOC], DT.bfloat16, tag="bpadE")
            bpadO = wpool.tile([128, BPAD_ALLOC], DT.bfloat16, tag="bpadO")
            nc.vector.memset(bpadE[:], 0.0)
            nc.vector.memset(bpadO[:], 0.0)

            # stats tiles (zeroed so unused rows stay finite)
            st1 = spool.tile([128, 2], DT.float32)
            st2 = spool.tile([128, 8], DT.float32)
            st3 = spool.tile([128, 8], DT.float32)
            nc.vector.memset(st1[:], 0.0)
            nc.vector.memset(st2[:], 0.0)
            nc.vector.memset(st3[:], 0.0)

            # ---- conv21: y[(f,cm), pix] ----
            psum_y = pspool.tile([128, NCH, 512], DT.float32, tag="bigps")
            for p in range(4):
                for ch in range(NCH):
                    nc.tensor.matmul(
                        psum_y[32 * p:32 * p + 32, ch, 0:CHUNK],
                        w21bd[:], x_all[:, p * P + ch * CHUNK:p * P + (ch + 1) * CHUNK],
                        start=True, stop=True, tile_position=(0, 32 * p))

            # drain y (pre-BN) in one Act op, accumulating the sum
            y_sb = wpool.tile([128, P], DT.bfloat16, tag="y")
            y_v = y_sb[:].rearrange("p (c x) -> p c x", c=NCH, x=CHUNK)
            nc.scalar.activation(y_v, psum_y[:, :, 0:CHUNK], COPY,
                                 accum_out=st1[:, 0:1])
            # y sumsq on DVE (fast path to AR1)
            trash = tpool.tile([128, P], DT.bfloat16, tag="trash")
            nc.vector.scalar_tensor_tensor(trash[:], y_sb[:], 1.0, y_sb[:],
                                           op0=MULT, op1=MULT,
                                           accum_out=st1[:, 1:2])

            # AllReduce #1 (launch now; latency hidden by b-build below)
            cc1i = dpool.tile([128, 2], DT.float32)
            cc1o = dpool.tile([128, 2], DT.float32, addr_space="Shared")
            nc.sync.dma_start(cc1i[:], st1[:])
            nc.gpsimd.collective_compute(
                "AllReduce", ADD, replica_groups=[list(range(N_CORES))],
                ins=[cc1i.opt()], outs=[cc1o.opt()])
            ar1r = spool.tile([128, 2], DT.float32)
            nc.sync.dma_start(ar1r[:], cc1o[:])

            # ---- b = temporal shift of y, via PE permutation matmul ----
            psum_b = pspool.tile([128, NCH, 512], DT.float32, tag="bigps",
                                 name="psb")
            for ch in range(NCH):
                nc.tensor.matmul(
                    psum_b[:, ch, 0:CHUNK], shiftm[:],
                    y_sb[:, ch * CHUNK:(ch + 1) * CHUNK],
                    start=True, stop=True)
            # drain b into the padded interior: rows y=8*ch+u, cols x
            base = 3 * WPAD + 3
            # 4D view [p, ch(7), yy(8), x(56)] with strides (8*62, 62, 1)
            bdst = bpadE[:, base:base + 56 * WPAD].rearrange(
                "p (c y x) -> p c y x", c=NCH, y=8, x=WPAD)[:, :, :, 0:56]
            bsrc = psum_b[:, :, 0:CHUNK].rearrange(
                "p c (y x) -> p c y x", y=8, x=56)
            nc.scalar.activation(bdst, bsrc, COPY)

            # ---- BN21 coefficients ----
            def bn_vectors(npart, psum_st, gvec, bvec, pool):
                mean = pool.tile([npart, 1], DT.float32, name=f"mean{nc.next_id()}")
                e2 = pool.tile([npart, 1], DT.float32, name=f"e2{nc.next_id()}")
                var = pool.tile([npart, 1], DT.float32, name=f"var{nc.next_id()}")
                std = pool.tile([npart, 1], DT.float32, name=f"std{nc.next_id()}")
                rstd = pool.tile([npart, 1], DT.float32, name=f"rstd{nc.next_id()}")
                svec = pool.tile([npart, 1], DT.float32, name=f"svec{nc.next_id()}")
                tv = pool.tile([npart, 1], DT.float32, name=f"tv{nc.next_id()}")
                tvec = pool.tile([npart, 1], DT.float32, name=f"tvec{nc.next_id()}")
                eps_t = pool.tile([npart, 1], DT.float32, name=f"eps{nc.next_id()}")
                nc.vector.memset(eps_t[:], EPS)
                nc.scalar.mul(mean[:], psum_st[:, 0:1], 1.0 / NTOT)
                nc.scalar.mul(e2[:], psum_st[:, 1:2], 1.0 / NTOT)
                nc.vector.tensor_mul(var[:], mean[:], mean[:])
                nc.vector.tensor_sub(var[:], e2[:], var[:])
                nc.scalar.activation(std[:], var[:], SQRT, bias=eps_t[:])
                nc.vector.reciprocal(rstd[:], std[:])
                nc.vector.tensor_mul(svec[:], gvec, rstd[:])
                nc.vector.tensor_mul(tv[:], mean[:], svec[:])
                nc.vector.tensor_sub(tvec[:], bvec, tv[:])
                return svec, tvec

            pst1 = pspool.tile([128, 2], DT.float32, tag="stps")
            nc.tensor.matmul(pst1[:], selb16[:], ar1r[:], start=True, stop=True)
            s21, t21 = bn_vectors(128, pst1, bnc128[:, 0:1], bnc128[:, 1:2], spool)

            # BN21 apply + relu on y (-> a) and on the b interior
            nc.scalar.activation(y_sb[:], y_sb[:], RELU, bias=t21[:], scale=s21[:])
            a_bf = y_sb
            bint = bpadE[:, base:base + 56 * WPAD].rearrange(
                "p (y x) -> p y x", y=56, x=WPAD)[:, :, 0:56]
            nc.scalar.activation(bint, bint, RELU, bias=t21[:], scale=s21[:])
            # odd-parity copy: bpadO[q] = bpadE[q+1]
            nc.sync.dma_start(bpadO[:, 0:BPAD_ALLOC - 1],
                              bpadE[:, 1:BPAD_ALLOC])

            dump("a", a_bf[:], [128, P], DT.bfloat16)
            dump("bpadE", bpadE[:], [128, BPAD_ALLOC], DT.bfloat16)
            dump("bpadO", bpadO[:], [128, BPAD_ALLOC], DT.bfloat16)

            # ---- correlation ----
            corr_all = bpool.tile([128, 4 * P], DT.bfloat16, tag="corr")
            corr2 = bpool.tile([98, 4 * P], DT.bfloat16, tag="corr2")
            a3dv = a_bf[:].rearrange("p (y x) -> p y x", y=56, x=56)
            k = 0
            for r, nslots in enumerate(ROUNDS):
                nrows = 128 if nslots == 16 else 32
                psum_corr = pspool.tile([128, NCH, 512], DT.float32, tag="bigps",
                                        name=f"psc{r}")
                for s in range(nslots):
                    dy, dx = k // 7, k % 7
                    delta = WPAD * dy + dx
                    src, off = (bpadE, delta) if delta % 2 == 0 else (bpadO, delta - 1)
                    bview = src[:, off:off + 56 * WPAD]
                    bview = bview.rearrange("p (y x) -> p y x", y=56, x=WPAD)
                    bview = bview[:, :, 0:56]
                    prod = ppool.tile([128, P], DT.bfloat16, tag="prod",
                                      name=f"prod{k}")
                    p3d = prod[:].rearrange("p (y x) -> p y x", y=56, x=56)
                    nc.vector.tensor_mul(p3d, a3dv, bview)
                    t4 = 32 * (s // 4)
                    for ch in range(NCH):
                        nc.tensor.matmul(
                            psum_corr[t4:t4 + 32, ch, 0:CHUNK],
                            sel32[:, 32 * (s % 4):32 * (s % 4) + 32],
                            prod[:, ch * CHUNK:(ch + 1) * CHUNK],
                            start=(s % 4 == 0), stop=(s % 4 == 3 or s == nslots - 1),
                            tile_position=(0, t4))
                    k += 1
                # drain round r (+ BN22 sum via accum)
                csl = slice(r * P, (r + 1) * P)
                corr_v = corr_all[0:nrows, csl].rearrange(
                    "p (c x) -> p c x", c=NCH, x=CHUNK)
                nc.scalar.activation(corr_v, psum_corr[0:nrows, :, 0:CHUNK], COPY,
                                     accum_out=st2[0:nrows, r:r + 1])
                # sumsq: last round on DVE (critical for AR2), others on Pool
                trash2 = tpool.tile([128, P], DT.bfloat16, tag="trash",
                                    name=f"trash2_{r}")
                nc.vector.scalar_tensor_tensor(
                    trash2[0:nrows, :], corr_all[0:nrows, csl], 1.0,
                    corr_all[0:nrows, csl],
                    op0=MULT, op1=MULT, accum_out=st2[0:nrows, 4 + r:5 + r])
                # per-round re-layout into conv22 operand order
                for f in range(F):
                    src2 = corr_all[f:8 * (nslots - 1) + f + 1:8, csl]
                    dst2 = corr2[49 * (f % 2) + 16 * r:
                                 49 * (f % 2) + 16 * r + nslots,
                                 (f // 2) * P:(f // 2 + 1) * P]
                    nc.sync.dma_start(dst2, src2)

            dump("corr", corr_all[:], [128, 4 * P], DT.bfloat16)
            # AllReduce #2
            cc2i = dpool.tile([128, 8], DT.float32)
            cc2o = dpool.tile([128, 8], DT.float32, addr_space="Shared")
            nc.sync.dma_start(cc2i[:], st2[:])
            nc.gpsimd.collective_compute(
                "AllReduce", ADD, replica_groups=[list(range(N_CORES))],
                ins=[cc2i.opt()], outs=[cc2o.opt()])
            ar2r = spool.tile([128, 8], DT.float32)
            nc.sync.dma_start(ar2r[:], cc2o[:])

            # BN22 vectors in (f2,k) layout
            pst2 = pspool.tile([98, 2], DT.float32, tag="stps", name="pst2")
            ar2v = ar2r[:].rearrange("p (s r) -> p r s", s=2, r=4)
            for r in range(4):
                nc.tensor.matmul(pst2[:], selbk[:, 98 * r:98 * (r + 1)],
                                 ar2v[:, r, :], start=(r == 0), stop=(r == 3))
            s22, t22 = bn_vectors(98, pst2, bnc98[:, 0:1], bnc98[:, 1:2], spool)

            # fold s22 into conv22 weights; bias b22v = t22/s22
            rs22 = spool.tile([98, 1], DT.float32)
            b22v = spool.tile([98, 1], DT.float32)
            nc.vector.reciprocal(rs22[:], s22[:])
            nc.vector.tensor_mul(b22v[:], t22[:], rs22[:])
            w22f = spool.tile([98, 128], DT.bfloat16)
            nc.vector.tensor_scalar(w22f[:], w22bd[:], s22[:], None, op0=MULT)

            # BN22 apply: relu(corr + b22v), split Act / DVE
            nc.scalar.activation(corr2[:, 0:2 * P], corr2[:, 0:2 * P], RELU,
                                 bias=b22v[:])
            nc.vector.tensor_scalar(corr2[:, 2 * P:4 * P], corr2[:, 2 * P:4 * P],
                                    b22v[:], 0.0, op0=ADD, op1=MAX)

            dump("corr2post", corr2[:], [98, 4 * P], DT.bfloat16)
            # ---- conv22 ----
            z_all = bpool.tile([128, 4 * P], DT.bfloat16, tag="z")
            for p in range(4):
                psum_z = pspool.tile([128, NCH, 512], DT.float32, tag="bigps",
                                     name=f"psz{p}")
                for ch in range(NCH):
                    nc.tensor.matmul(
                        psum_z[:, ch, 0:CHUNK], w22f[:],
                        corr2[:, p * P + ch * CHUNK:p * P + (ch + 1) * CHUNK],
                        start=True, stop=True)
                zsl = slice(p * P, (p + 1) * P)
                z_v = z_all[:, zsl].rearrange("p (c x) -> p c x", c=NCH, x=CHUNK)
                nc.scalar.activation(z_v, psum_z[:, :, 0:CHUNK], COPY,
                                     accum_out=st3[:, p:p + 1])
                trash3 = tpool.tile([128, P], DT.bfloat16, tag="trash",
                                    name=f"trash3_{p}")
                nc.vector.scalar_tensor_tensor(
                    trash3[:], z_all[:, zsl], 1.0, z_all[:, zsl],
                    op0=MULT, op1=MULT, accum_out=st3[:, 4 + p:5 + p])

            ar3 = spool.tile([128, 2], DT.float32)
            nc.vector.tensor_reduce(ar3[:, 0:1], st3[:, 0:4],
                                    axis=mybir.AxisListType.X, op=ADD)
            nc.vector.tensor_reduce(ar3[:, 1:2], st3[:, 4:8],
                                    axis=mybir.AxisListType.X, op=ADD)

            # AllReduce #3
            cc3i = dpool.tile([128, 2], DT.float32)
            cc3o = dpool.tile([128, 2], DT.float32, addr_space="Shared")
            nc.sync.dma_start(cc3i[:], ar3[:])
            nc.gpsimd.collective_compute(
                "AllReduce", ADD, replica_groups=[list(range(N_CORES))],
                ins=[cc3i.opt()], outs=[cc3o.opt()])
            ar3r = spool.tile([128, 2], DT.float32)
            nc.sync.dma_start(ar3r[:], cc3o[:])

            pst3 = pspool.tile([128, 2], DT.float32, tag="stps", name="pst3")
            nc.tensor.matmul(pst3[:], selb64[:], ar3r[:], start=True, stop=True)
            s23, t23 = bn_vectors(128, pst3, bnc128[:, 2:3], bnc128[:, 3:4], spool)

            dump("z", z_all[:], [128, 4 * P], DT.bfloat16)
            # ---- final: relu(s23*z + t23 + x), bf16 out ----
            for p in range(4):
                zsl = slice(p * P, (p + 1) * P)
                tmp = ppool.tile([128, P], DT.bfloat16, tag="prod",
                                 name=f"fin{p}")
                nc.vector.scalar_tensor_tensor(
                    tmp[:], z_all[:, zsl], s23[:], x_all[:, zsl],
                    op0=MULT, op1=ADD)
                o16 = wpool.tile([128, P], DT.bfloat16, tag="o16",
                                 name=f"o16_{p}")
                nc.scalar.activation(o16[:], tmp[:], RELU, bias=t23[:])
                nc.sync.dma_start(out_d[p], o16[:])

    nc.compile()
    nc._dbg_names = list(dbg_tensors)
    return nc


def _host_constants(w21, w22):
    w21bd = np.zeros((128, 32), BF16)
    for f2 in range(2):
        w21bd[64 * f2:64 * f2 + 64, 16 * f2:16 * f2 + 16] = w21.T.astype(BF16)
    w22bd = np.zeros((98, 128), BF16)
    for f2 in range(2):
        w22bd[49 * f2:49 * f2 + 49, 64 * f2:64 * f2 + 64] = w22.T.astype(BF16)

    # corr reduce stationaries: 4 variants (m = s%4), each [128, 32];
    # variant m maps product row (f,cm) -> within-tile psum row 8m+f
    sel32 = np.zeros((128, 4 * 32), BF16)
    for m in range(4):
        for f in range(F):
            sel32[16 * f:16 * f + 16, 32 * m + 8 * m + f] = 1.0 / CM

    pidx = np.arange(128)
    selb16 = (pidx[:, None] % 16 == pidx[None, :] % 16).astype(np.float32)
    selb64 = (pidx[:, None] % 64 == pidx[None, :] % 64).astype(np.float32)

    selbk = np.zeros((4, 128, 98), np.float32)
    for r, nslots in enumerate(ROUNDS):
        for s in range(nslots):
            for f in range(F):
                for f2 in range(2):
                    selbk[r, 8 * s + f, 49 * f2 + 16 * r + s] = 1.0
    selbk = selbk.transpose(1, 0, 2).reshape(128, 4 * 98)

    shiftm = np.zeros((128, 128), BF16)
    for f in range(7):
        for cm in range(16):
            shiftm[16 * (f + 1) + cm, 16 * f + cm] = 1.0
    for cm in range(16):
        shiftm[112 + cm, 112 + cm] = 1.0
    return w21bd, w22bd, sel32, shiftm, selb16, selb64, selbk


_NC_CACHE = {}


def kernel(x, w21, w22, g21, b21, g22, b22, g23, b23, trace=False, dbg=False):
    x = np.asarray(x, np.float32)
    w21 = np.asarray(w21, np.float32)
    w22 = np.asarray(w22, np.float32)
    g21 = np.asarray(g21, np.float32); b21 = np.asarray(b21, np.float32)
    g22 = np.asarray(g22, np.float32); b22 = np.asarray(b22, np.float32)
    g23 = np.asarray(g23, np.float32); b23 = np.asarray(b23, np.float32)

    key = ("nc_dbg" if dbg else "nc")
    if key not in _NC_CACHE:
        _NC_CACHE[key] = _build_nc(dbg=dbg)
    nc = _NC_CACHE[key]

    w21bd, w22bd, sel32, shiftm, selb16, selb64, selbk = _host_constants(w21, w22)
    pidx = np.arange(128)
    bnc128 = np.stack([g21[pidx % 16], b21[pidx % 16],
                       g23[pidx % 64], b23[pidx % 64]], 1).astype(np.float32)
    kidx = np.arange(98) % 49
    bnc98 = np.stack([g22[kidx], b22[kidx]], 1).astype(np.float32)

    in_maps = []
    for i in range(N_CORES):
        x4 = np.ascontiguousarray(
            x[F * i:F * (i + 1)].reshape(4, 128, P)).astype(BF16)
        in_maps.append({
            "x4": x4, "w21bd": w21bd, "w22bd": w22bd, "sel32": sel32,
            "shiftm": shiftm, "selb16": selb16, "selb64": selb64,
            "selbk": selbk, "bnc128": bnc128, "bnc98": bnc98,
        })

    res = run_bass_kernel_spmd(nc, in_maps, core_ids=list(range(N_CORES)),
                               trace=trace)
    out = np.empty((NT, C, H, W), np.float32)
    for i in range(N_CORES):
        out[F * i:F * (i + 1)] = np.asarray(
            res.results[i]["out"]).astype(np.float32).reshape(F, C, H, W)
    if dbg or trace:
        return out, res
    return out
